# revision 22
# baseline (speedup 1.0000x reference)
"""Trainium2 Bass kernel for nn_AttentionLayer (B=2048, N=64, D=256, H=16).

Math (per batch row b):
  total = sum_n m[b,n,:];  diff = (N*m - total) * item[b]
  logits = relu(diff @ W1 + b1) @ W2;  out = softmax(logits over n)

Restructured:  G = (m*item) @ W1;  S = sum_n G = (total*item) @ W1;
  pre = 64*G - S + b1.

Layout: process macro-batches of 32 batch rows (2048 bn rows, 2 MB),
engine-balanced so everything hides under the members DMA (~5.8us/mb):
  - members loaded naturally [128 bn, d]; PE identity-transposes (f32r,
    1.5 cyc/row) into PSUM [d, bn] blocks.  Alongside each 128-col
    transpose, a 2-col transpose-mode matmul against a ones RHS emits
    the per-b totals sum_n m in d-major form (~1ns each on PE).
  - DVE multiplies by item (stride-0 broadcast over n) while moving
    PSUM -> SBUF (f32r 'scaled'), plus tiny tot*item ops.
  - mm1 per (group, chunk): G[16h, 8b x 64n] PSUM, free 512 keeps f32r
    at full rate.  S = W1^T @ (tot*item) is a tiny PE matmul; the
    "-S/64" correction is a rank-16 accumulating matmul with stride-0
    rhs (baseline trick), then ACT applies relu(64*G + b1).
  - mm2 moves OFF the PE: ACT scales hrel by W2[h] (per-partition
    scale), and the otherwise-idle Pool engine partition_all_reduces
    the 16 h-rows into logits.
  - logits bounce through DRAM (trivial bytes) for the [128, 64]
    partition-major softmax phase.

Sharding: pure data-parallel over B across 8 cores (one SPMD NEFF).
"""
import numpy as np
from contextlib import ExitStack

import concourse.bacc as bacc
import concourse.tile as tile
import concourse.mybir as mybir
import concourse.bass_isa as bass_isa
from concourse import bass_utils

B, N, D, H = 2048, 64, 256, 16
NCORES = 8
BL = B // NCORES          # 256 batch rows per core
MB = 8                    # macro-batches per core
GBM = BL // MB            # 32 batch rows per macro-batch
ROWS = GBM * N            # 2048 bn rows per macro-batch
CH = D // 128             # 2 d chunks
NG4 = 4                   # groups of 8 batch rows per macro-batch
GB = 8                    # batch rows per group

f32 = mybir.dt.float32
f32r = mybir.dt.float32r
AF = mybir.ActivationFunctionType
OP = mybir.AluOpType

_cache = {}


def _build():
    nc = bacc.Bacc("TRN2", target_bir_lowering=False)
    mem = nc.dram_tensor("mem", [BL * N, D], f32r, kind="ExternalInput")
    itT = nc.dram_tensor("itT", [CH, 128, BL], f32, kind="ExternalInput")
    W1d = nc.dram_tensor("W1d", [D, H], f32r, kind="ExternalInput")
    b1d = nc.dram_tensor("b1d", [H, 1], f32, kind="ExternalInput")
    W2d = nc.dram_tensor("W2d", [H, 1], f32, kind="ExternalInput")
    idd = nc.dram_tensor("idd", [128, 128], f32r, kind="ExternalInput")
    on2 = nc.dram_tensor("on2", [128, 2], f32r, kind="ExternalInput")
    nId = nc.dram_tensor("nId", [H, H], f32r, kind="ExternalInput")
    out = nc.dram_tensor("out", [BL, N], f32, kind="ExternalOutput")
    lgs = nc.dram_tensor("lgs", [BL, N], f32, kind="Internal")

    with tile.TileContext(nc) as tc, ExitStack() as ctx:
        const = ctx.enter_context(tc.tile_pool(name="const", bufs=1))
        mpool = ctx.enter_context(tc.tile_pool(name="mpool", bufs=2))
        spool = ctx.enter_context(tc.tile_pool(name="spool", bufs=2))
        hpool = ctx.enter_context(tc.tile_pool(name="hpool", bufs=2))
        bpool = ctx.enter_context(tc.tile_pool(name="bpool", bufs=2))
        tps = ctx.enter_context(tc.tile_pool(name="tps", bufs=2, space="PSUM"))
        gps = ctx.enter_context(tc.tile_pool(name="gps", bufs=4, space="PSUM"))
        aux = ctx.enter_context(tc.tile_pool(name="aux", bufs=2, space="PSUM"))

        ident = const.tile([128, 128], f32r)
        ones2 = const.tile([128, 2], f32r)
        itT_sb = const.tile([128, CH, BL], f32)
        W1_sb = const.tile([128, CH, H], f32r)
        b1_sb = const.tile([H, 1], f32)
        W2_sb = const.tile([H, 1], f32)
        nI_sb = const.tile([H, H], f32r)
        nc.gpsimd.dma_start(out=ident, in_=idd[:, :])
        nc.gpsimd.dma_start(out=ones2, in_=on2[:, :])
        nc.gpsimd.dma_start(
            out=itT_sb, in_=itT[:, :, :].rearrange("c p b -> p c b"))
        nc.gpsimd.dma_start(
            out=W1_sb, in_=W1d[:, :].rearrange("(c p) h -> p c h", p=128))
        nc.gpsimd.dma_start(out=b1_sb, in_=b1d[:, :])
        nc.gpsimd.dma_start(out=W2_sb, in_=W2d[:, :])
        nc.gpsimd.dma_start(out=nI_sb, in_=nId[:, :])

        p2 = ctx.enter_context(tc.tile_pool(name="p2", bufs=2))

        def softmax_tile(t):
            # softmax over n in [b, n] partition-major layout
            lg_t = p2.tile([128, N], f32, name="lg_t")
            nc.gpsimd.dma_start(out=lg_t, in_=lgs[t * 128:(t + 1) * 128, :])
            nmax = p2.tile([128, 1], f32, name="nmax")
            nc.vector.tensor_reduce(nmax[:], lg_t[:], axis=mybir.AxisListType.X,
                                    op=OP.max, negate=True)
            ex = p2.tile([128, N], f32, name="ex")
            ssum = p2.tile([128, 1], f32, name="ssum")
            nc.scalar.activation(ex[:], lg_t[:], AF.Exp, bias=nmax[:],
                                 scale=1.0, accum_out=ssum[:])
            rs = p2.tile([128, 1], f32, name="rs")
            nc.vector.reciprocal(rs[:], ssum[:])
            o_t = p2.tile([128, N], f32, name="o_t")
            nc.vector.tensor_scalar_mul(o_t[:], ex[:], rs[:])
            nc.gpsimd.dma_start(out=out[t * 128:(t + 1) * 128, :], in_=o_t[:])

        # Software-pipelined: iteration `it` PRODUCES scaled/tot/S for
        # macro-batch `it` and CONSUMES macro-batch `it-1` (mm1 onward),
        # so the PE never stalls on the cross-engine S/relu chain.
        state = {}
        for it in range(MB + 1):
            if it < MB:
                mb = it
                # natural-layout load: [128 bn, blk, chunk, d] (1KB runs)
                m4 = mpool.tile([128, 16, CH, 128], f32r)
                src = mem[mb * ROWS:(mb + 1) * ROWS, :].rearrange(
                    "(blk p) (c d) -> p blk c d", p=128, c=CH)
                nc.sync.dma_start(out=m4, in_=src)

                # scaled4[d, c, u=(g b), n] = m * item;
                # comb packs tot (per-b totals, d-major) + S in one bank
                scaled4 = spool.tile([128, CH, GBM, N], f32r)
                comb = aux.tile([128, 3, GBM], f32, tag="comb")
                tot_it = bpool.tile([128, CH, GBM], f32r)
                for c in range(CH):
                    for q in range(4):
                        # PE: 4 transposes + 4 tiny totals matmuls -> PSUM
                        # (tot is a REGULAR matmul vs the ones RHS)
                        T4 = tps.tile([128, 4, 128], f32r)
                        for j in range(4):
                            i = q * 4 + j
                            nc.tensor.transpose(
                                T4[:, j, :], m4[:, i, c, :], ident[:])
                            nc.tensor.matmul(
                                comb[:, c, 2 * i:2 * i + 2],
                                m4[:, i, c, :], ones2[:],
                                start=True, stop=True)
                        # DVE: item multiply + PSUM->SBUF move (512 cols)
                        u0 = mb * GBM + q * 8
                        nc.vector.tensor_mul(
                            scaled4[:, c, q * 8:q * 8 + 8, :],
                            T4[:, :, :].bitcast(f32).rearrange(
                                "p j (x n) -> p (j x) n", x=2),
                            itT_sb[:, c, u0:u0 + 8].unsqueeze(2).broadcast_to(
                                [128, 8, N]))
                    nc.vector.tensor_mul(
                        tot_it[:, c, :], comb[:, c, :],
                        itT_sb[:, c, mb * GBM:(mb + 1) * GBM])

                # S[h, u] = W1^T @ tot_it (tiny PE matmuls) -> SBUF f32r
                for c in range(CH):
                    nc.tensor.matmul(comb[0:H, 2, :], W1_sb[:, c, :],
                                     tot_it[:, c, :],
                                     start=(c == 0), stop=(c == CH - 1))
                S_sb = bpool.tile([H, GBM], f32r, tag="S_sb")
                nc.scalar.copy(S_sb[:, :], comb[0:H, 2, :])
                state[mb] = (scaled4, S_sb)

            if it >= 1:
                mb = it - 1
                scaled4, S_sb = state.pop(mb)
                # per group: mm1 (2 chunks) + rank-16 correction -S/64
                hrel = hpool.tile([H, NG4, GB, N], f32r, tag="hrel")
                w2h = hpool.tile([H, NG4, GB, N], f32, tag="w2h")
                lgr = hpool.tile([H, NG4, GB, N], f32, tag="lgr")
                for g in range(NG4):
                    G = gps.tile([H, GB, N], f32)
                    for c in range(CH):
                        nc.tensor.matmul(
                            G[:, :, :], W1_sb[:, c, :],
                            scaled4[:, c, g * GB:(g + 1) * GB, :],
                            start=(c == 0), stop=False)
                    nc.tensor.matmul(
                        G[:, :, :], nI_sb[:],
                        S_sb[:, g * GB:(g + 1) * GB].unsqueeze(2).broadcast_to(
                            [H, GB, N]),
                        start=False, stop=True)
                    # relu(64*(G - S/64) + b1) -> hrel;  then * W2[h] on ACT
                    nc.scalar.activation(hrel[:, g, :, :], G[:, :, :], AF.Relu,
                                         bias=b1_sb[:], scale=64.0)
                    nc.scalar.activation(w2h[:, g, :, :], hrel[:, g, :, :],
                                         AF.Copy, scale=W2_sb[:])
                    # logits[u, n] = sum_h w2h  on the Pool engine
                    nc.gpsimd.partition_all_reduce(
                        lgr[:, g, :, :], w2h[:, g, :, :], channels=H,
                        reduce_op=bass_isa.ReduceOp.add)

                # row 0 of the all-reduce holds the logits for 32 batch rows
                nc.sync.dma_start(
                    out=lgs[mb * GBM:(mb + 1) * GBM, :].rearrange(
                        "r n -> (r n)").unsqueeze(0),
                    in_=lgr[0:1, :, :, :].rearrange("p g b n -> p (g b n)"))
                if mb == 3:
                    softmax_tile(0)

        softmax_tile(1)

    nc.compile()
    return nc


def make_in_maps(members_embeds, item_embeds, W1, b1, W2, b2=None):
    members_embeds = np.ascontiguousarray(members_embeds, dtype=np.float32)
    item_embeds = np.ascontiguousarray(item_embeds, dtype=np.float32)
    W1 = np.ascontiguousarray(W1, dtype=np.float32)
    b1 = np.asarray(b1, dtype=np.float32).reshape(H, 1)
    W2 = np.asarray(W2, dtype=np.float32).reshape(H, 1)
    # b2 drops out of softmax entirely.

    idv = np.eye(128, dtype=np.float32)
    # ones RHS for the totals transpose-matmuls: col j sums rows of b-half j
    on2 = np.zeros((128, 2), dtype=np.float32)
    on2[:64, 0] = 1.0
    on2[64:, 1] = 1.0
    # correction subtracts S/64 (relu scale=64 multiplies (G - S/64))
    nIv = (-np.eye(H, dtype=np.float32) / 64.0)

    in_maps = []
    for k in range(NCORES):
        mem_k = members_embeds[k * BL:(k + 1) * BL].reshape(BL * N, D)
        it_k = item_embeds[k * BL:(k + 1) * BL]            # [BL, D]
        itT_k = np.ascontiguousarray(
            it_k.T.reshape(CH, 128, BL), dtype=np.float32)
        in_maps.append({
            "mem": np.ascontiguousarray(mem_k),
            "itT": itT_k,
            "W1d": W1, "b1d": b1, "W2d": W2,
            "idd": idv, "on2": on2, "nId": nIv,
        })

    return in_maps


def kernel(members_embeds, item_embeds, W1, b1, W2, b2=None):
    if "nc" not in _cache:
        _cache["nc"] = _build()
    nc = _cache["nc"]
    in_maps = make_in_maps(members_embeds, item_embeds, W1, b1, W2, b2)
    res = bass_utils.run_bass_kernel_spmd(
        nc, in_maps, core_ids=list(range(NCORES)))
    return np.concatenate([r["out"] for r in res.results], axis=0)


# revision 23
# speedup vs baseline: 1.0571x; 1.0571x over previous
"""Trainium2 Bass kernel for nn_AttentionLayer (B=2048, N=64, D=256, H=16).

Math (per batch row b):
  total = sum_n m[b,n,:];  diff = (N*m - total) * item[b]
  logits = relu(diff @ W1 + b1) @ W2;  out = softmax(logits over n)

Restructured:  G = (m*item) @ W1;  S = sum_n G = (total*item) @ W1;
  pre = 64*G - S + b1.

Layout: process macro-batches of 32 batch rows (2048 bn rows, 2 MB),
engine-balanced so everything hides under the members DMA (~5.8us/mb):
  - members loaded naturally [128 bn, d]; PE identity-transposes (f32r,
    1.5 cyc/row) into PSUM [d, bn] blocks.  Alongside each 128-col
    transpose, a 2-col transpose-mode matmul against a ones RHS emits
    the per-b totals sum_n m in d-major form (~1ns each on PE).
  - DVE multiplies by item (stride-0 broadcast over n) while moving
    PSUM -> SBUF (f32r 'scaled'), plus tiny tot*item ops.
  - mm1 per (group, chunk): G[16h, 8b x 64n] PSUM, free 512 keeps f32r
    at full rate.  S = W1^T @ (tot*item) is a tiny PE matmul; the
    "-S/64" correction is a rank-16 accumulating matmul with stride-0
    rhs (baseline trick), then ACT applies relu(64*G + b1).
  - mm2 moves OFF the PE: ACT scales hrel by W2[h] (per-partition
    scale), and the otherwise-idle Pool engine partition_all_reduces
    the 16 h-rows into logits.
  - logits bounce through DRAM (trivial bytes) for the [128, 64]
    partition-major softmax phase.

Sharding: pure data-parallel over B across 8 cores (one SPMD NEFF).
"""
import numpy as np
from contextlib import ExitStack

import concourse.bacc as bacc
import concourse.tile as tile
import concourse.mybir as mybir
import concourse.bass_isa as bass_isa
from concourse import bass_utils

B, N, D, H = 2048, 64, 256, 16
NCORES = 8
BL = B // NCORES          # 256 batch rows per core
MB = 8                    # macro-batches per core
GBM = BL // MB            # 32 batch rows per macro-batch
ROWS = GBM * N            # 2048 bn rows per macro-batch
CH = D // 128             # 2 d chunks
NG4 = 4                   # groups of 8 batch rows per macro-batch
GB = 8                    # batch rows per group

f32 = mybir.dt.float32
f32r = mybir.dt.float32r
AF = mybir.ActivationFunctionType
OP = mybir.AluOpType

_cache = {}


def _build():
    nc = bacc.Bacc("TRN2", target_bir_lowering=False)
    mem = nc.dram_tensor("mem", [BL * N, D], f32r, kind="ExternalInput")
    itT = nc.dram_tensor("itT", [CH, 128, BL], f32, kind="ExternalInput")
    W1d = nc.dram_tensor("W1d", [D, H], f32r, kind="ExternalInput")
    b1d = nc.dram_tensor("b1d", [H, 1], f32, kind="ExternalInput")
    W2d = nc.dram_tensor("W2d", [H, 1], f32, kind="ExternalInput")
    idd = nc.dram_tensor("idd", [128, 128], f32r, kind="ExternalInput")
    on2 = nc.dram_tensor("on2", [128, 2], f32r, kind="ExternalInput")
    nId = nc.dram_tensor("nId", [H, H], f32r, kind="ExternalInput")
    out = nc.dram_tensor("out", [BL, N], f32, kind="ExternalOutput")
    lgs = nc.dram_tensor("lgs", [BL, N], f32, kind="Internal")

    with tile.TileContext(nc) as tc, ExitStack() as ctx:
        const = ctx.enter_context(tc.tile_pool(name="const", bufs=1))
        mpool = ctx.enter_context(tc.tile_pool(name="mpool", bufs=2))
        spool = ctx.enter_context(tc.tile_pool(name="spool", bufs=2))
        hpool = ctx.enter_context(tc.tile_pool(name="hpool", bufs=2))
        bpool = ctx.enter_context(tc.tile_pool(name="bpool", bufs=2))
        tps = ctx.enter_context(tc.tile_pool(name="tps", bufs=2, space="PSUM"))
        gps = ctx.enter_context(tc.tile_pool(name="gps", bufs=4, space="PSUM"))
        aux = ctx.enter_context(tc.tile_pool(name="aux", bufs=2, space="PSUM"))

        ident = const.tile([128, 128], f32r)
        ones2 = const.tile([128, 2], f32r)
        itT_sb = const.tile([128, CH, BL], f32)
        W1_sb = const.tile([128, CH, H], f32r)
        b1_sb = const.tile([H, 1], f32)
        W2_sb = const.tile([H, 1], f32)
        nI_sb = const.tile([H, H], f32r)
        nc.gpsimd.dma_start(out=ident, in_=idd[:, :])
        nc.gpsimd.dma_start(out=ones2, in_=on2[:, :])
        nc.gpsimd.dma_start(
            out=itT_sb, in_=itT[:, :, :].rearrange("c p b -> p c b"))
        nc.gpsimd.dma_start(
            out=W1_sb, in_=W1d[:, :].rearrange("(c p) h -> p c h", p=128))
        nc.gpsimd.dma_start(out=b1_sb, in_=b1d[:, :])
        nc.gpsimd.dma_start(out=W2_sb, in_=W2d[:, :])
        nc.gpsimd.dma_start(out=nI_sb, in_=nId[:, :])

        p2 = ctx.enter_context(tc.tile_pool(name="p2", bufs=2))

        def softmax_tile(t):
            # softmax over n in [b, n] partition-major layout
            lg_t = p2.tile([128, N], f32, name="lg_t")
            nc.gpsimd.dma_start(out=lg_t, in_=lgs[t * 128:(t + 1) * 128, :])
            nmax = p2.tile([128, 1], f32, name="nmax")
            nc.vector.tensor_reduce(nmax[:], lg_t[:], axis=mybir.AxisListType.X,
                                    op=OP.max, negate=True)
            ex = p2.tile([128, N], f32, name="ex")
            ssum = p2.tile([128, 1], f32, name="ssum")
            nc.scalar.activation(ex[:], lg_t[:], AF.Exp, bias=nmax[:],
                                 scale=1.0, accum_out=ssum[:])
            rs = p2.tile([128, 1], f32, name="rs")
            nc.vector.reciprocal(rs[:], ssum[:])
            o_t = p2.tile([128, N], f32, name="o_t")
            nc.vector.tensor_scalar_mul(o_t[:], ex[:], rs[:])
            nc.gpsimd.dma_start(out=out[t * 128:(t + 1) * 128, :], in_=o_t[:])

        # Software-pipelined: iteration `it` PRODUCES scaled/tot for
        # macro-batch `it` and CONSUMES macro-batch `it-1` (S, mm1
        # onward).  The consume-phase PE matmuls are interleaved between
        # transpose quarters so the PE fills its T4-buffer waits instead
        # of stalling, and never waits on same-iteration DVE results.
        state = {}
        for it in range(MB + 1):
            produce = it < MB
            consume = it >= 1

            if produce:
                mb = it
                # natural-layout load: [128 bn, blk, chunk, d] (1KB runs)
                m4 = mpool.tile([128, 16, CH, 128], f32r)
                src = mem[mb * ROWS:(mb + 1) * ROWS, :].rearrange(
                    "(blk p) (c d) -> p blk c d", p=128, c=CH)
                nc.sync.dma_start(out=m4, in_=src)
                scaled4 = spool.tile([128, CH, GBM, N], f32r)
                tot_it = bpool.tile([128, CH, GBM], f32r)
            # comb packs tot (per-b totals, d-major) + last mb's S psum
            comb = aux.tile([128, 3, GBM], f32, tag="comb")

            units = []
            if consume:
                mbc = it - 1
                scaled4c, tot_itc = state.pop(mbc)
                hrel = hpool.tile([H, NG4, GB, N], f32r, tag="hrel")
                w2h = hpool.tile([H, NG4, GB, N], f32, tag="w2h")
                lgr = hpool.tile([H, NG4, GB, N], f32, tag="lgr")
                sref = {}

                def s_unit(tot_itc=tot_itc, comb=comb, sref=sref):
                    # S[h, u] = W1^T @ tot_it (tiny PE matmuls) -> SBUF
                    for c in range(CH):
                        nc.tensor.matmul(comb[0:H, 2, :], W1_sb[:, c, :],
                                         tot_itc[:, c, :],
                                         start=(c == 0), stop=(c == CH - 1))
                    S_sb = bpool.tile([H, GBM], f32r, tag="S_sb",
                                      name="S_sb")
                    nc.scalar.copy(S_sb[:, :], comb[0:H, 2, :])
                    sref["S"] = S_sb

                units.append(s_unit)

                def g_unit(g, scaled4c=scaled4c, sref=sref, hrel=hrel,
                           w2h=w2h, lgr=lgr):
                    # mm1 (2 chunks) + rank-16 correction -S/64
                    G = gps.tile([H, GB, N], f32, name="G")
                    for c in range(CH):
                        nc.tensor.matmul(
                            G[:, :, :], W1_sb[:, c, :],
                            scaled4c[:, c, g * GB:(g + 1) * GB, :],
                            start=(c == 0), stop=False)
                    nc.tensor.matmul(
                        G[:, :, :], nI_sb[:],
                        sref["S"][:, g * GB:(g + 1) * GB].unsqueeze(
                            2).broadcast_to([H, GB, N]),
                        start=False, stop=True)
                    # relu(64*(G - S/64) + b1) -> hrel; then * W2[h] on ACT
                    nc.scalar.activation(hrel[:, g, :, :], G[:, :, :],
                                         AF.Relu, bias=b1_sb[:], scale=64.0)
                    nc.scalar.activation(w2h[:, g, :, :], hrel[:, g, :, :],
                                         AF.Copy, scale=W2_sb[:])
                    # logits[u, n] = sum_h w2h  on the Pool engine
                    nc.gpsimd.partition_all_reduce(
                        lgr[:, g, :, :], w2h[:, g, :, :], channels=H,
                        reduce_op=bass_isa.ReduceOp.add)

                for g in range(NG4):
                    units.append(lambda g=g: g_unit(g))

            if produce:
                mb = it
                emitted = 0
                for qi, (c, q) in enumerate(
                        [(c, q) for c in range(CH) for q in range(4)]):
                    # PE: 4 transposes + 4 tiny totals matmuls -> PSUM
                    # (tot is a REGULAR matmul vs the ones RHS)
                    T4 = tps.tile([128, 4, 128], f32r)
                    for j in range(4):
                        i = q * 4 + j
                        nc.tensor.transpose(
                            T4[:, j, :], m4[:, i, c, :], ident[:])
                        nc.tensor.matmul(
                            comb[:, c, 2 * i:2 * i + 2],
                            m4[:, i, c, :], ones2[:],
                            start=True, stop=True)
                    # DVE: item multiply + PSUM->SBUF move (512 cols)
                    u0 = mb * GBM + q * 8
                    nc.vector.tensor_mul(
                        scaled4[:, c, q * 8:q * 8 + 8, :],
                        T4[:, :, :].bitcast(f32).rearrange(
                            "p j (x n) -> p (j x) n", x=2),
                        itT_sb[:, c, u0:u0 + 8].unsqueeze(2).broadcast_to(
                            [128, 8, N]))
                    if q == 3:
                        nc.vector.tensor_mul(
                            tot_it[:, c, :], comb[:, c, :],
                            itT_sb[:, c, mb * GBM:(mb + 1) * GBM])
                    # interleave consume-phase PE work into T4 waits
                    want = (qi + 1) * len(units) // 8
                    while emitted < want:
                        units[emitted]()
                        emitted += 1
                while emitted < len(units):
                    units[emitted]()
                    emitted += 1
                state[mb] = (scaled4, tot_it)
            else:
                for u in units:
                    u()

            if consume:
                mbc = it - 1
                # row 0 of the all-reduce holds the logits for 32 rows
                nc.sync.dma_start(
                    out=lgs[mbc * GBM:(mbc + 1) * GBM, :].rearrange(
                        "r n -> (r n)").unsqueeze(0),
                    in_=lgr[0:1, :, :, :].rearrange("p g b n -> p (g b n)"))
                if mbc == 3:
                    softmax_tile(0)

        softmax_tile(1)

    nc.compile()
    return nc


def make_in_maps(members_embeds, item_embeds, W1, b1, W2, b2=None):
    members_embeds = np.ascontiguousarray(members_embeds, dtype=np.float32)
    item_embeds = np.ascontiguousarray(item_embeds, dtype=np.float32)
    W1 = np.ascontiguousarray(W1, dtype=np.float32)
    b1 = np.asarray(b1, dtype=np.float32).reshape(H, 1)
    W2 = np.asarray(W2, dtype=np.float32).reshape(H, 1)
    # b2 drops out of softmax entirely.

    idv = np.eye(128, dtype=np.float32)
    # ones RHS for the totals transpose-matmuls: col j sums rows of b-half j
    on2 = np.zeros((128, 2), dtype=np.float32)
    on2[:64, 0] = 1.0
    on2[64:, 1] = 1.0
    # correction subtracts S/64 (relu scale=64 multiplies (G - S/64))
    nIv = (-np.eye(H, dtype=np.float32) / 64.0)

    in_maps = []
    for k in range(NCORES):
        mem_k = members_embeds[k * BL:(k + 1) * BL].reshape(BL * N, D)
        it_k = item_embeds[k * BL:(k + 1) * BL]            # [BL, D]
        itT_k = np.ascontiguousarray(
            it_k.T.reshape(CH, 128, BL), dtype=np.float32)
        in_maps.append({
            "mem": np.ascontiguousarray(mem_k),
            "itT": itT_k,
            "W1d": W1, "b1d": b1, "W2d": W2,
            "idd": idv, "on2": on2, "nId": nIv,
        })

    return in_maps


def kernel(members_embeds, item_embeds, W1, b1, W2, b2=None):
    if "nc" not in _cache:
        _cache["nc"] = _build()
    nc = _cache["nc"]
    in_maps = make_in_maps(members_embeds, item_embeds, W1, b1, W2, b2)
    res = bass_utils.run_bass_kernel_spmd(
        nc, in_maps, core_ids=list(range(NCORES)))
    return np.concatenate([r["out"] for r in res.results], axis=0)
